# revision 18
# baseline (speedup 1.0000x reference)
"""Trainium2 Bass kernel for nn_DPFA_29454885716026 (deformable conv + pos attention).

Strategy: data-parallel over batch B=8 across the 8 NeuronCores (one sample per
core).  Self-contained: builds the Bass/Tile program once (module cache), shards
inputs, runs via bass_utils.run_bass_kernel_spmd, reassembles the full output.

Per-core pipeline (sample x_b [128, 64, 64]):
  PE : offset conv 128->27 (3x3) | om transposes to pixel-partition |
       9-tap block-diag deform matmuls | position-attention matmuls
  ACT: relu/sigmoid evacuations
  DVE: bilinear weight pipeline (pixel-partition) | weight-apply + corner sums
  GPSIMD: ap_gather (4 corners x 9 taps, channel-partition)
  DMA: loads, DRAM bounces for idx/weight layout changes, weight broadcast
"""

import numpy as np
import ml_dtypes
from contextlib import ExitStack

import concourse.bass as bass
import concourse.mybir as mybir
import concourse.tile as tile
from concourse import bacc
from concourse._compat import with_exitstack
from concourse.bass_utils import run_bass_kernel_spmd

F32 = mybir.dt.float32
BF16 = mybir.dt.bfloat16
I16 = mybir.dt.int16
BF16_NP = ml_dtypes.bfloat16

B = 8
C = 128
H = W = 64
HW = H * W
NPIX_PAD = 4224
K = 9
NT = 32
CHUNK = 1024
NCHUNK = HW // CHUNK
AluOp = mybir.AluOpType
ActFn = mybir.ActivationFunctionType

N_CORES = 8
OMCH = 41   # om psum channels: 0-17 offsets, 32-40 masks (18-31 unused)


def rap(t, extra_off, dims):
    """Raw AP (DRAM tiles: all dims are plain strides)."""
    return bass.AP(tensor=t.tensor, offset=t.offset + extra_off,
                   ap=[list(d) for d in dims])


def rsb(t, extra_off, free_dims):
    """SBUF tile view: keep the tile's partition dim, replace free dims."""
    base = t[:]
    return bass.AP(tensor=base.tensor, offset=base.offset + extra_off,
                   ap=[list(base.ap[0])] + [list(d) for d in free_dims])


@with_exitstack
def dpfa_kernel(ctx: ExitStack, tc: tile.TileContext, outs, ins):
    nc = tc.nc
    out_dram = outs["out"]

    def dbg(name, ap):
        if name in outs:
            nc.sync.dma_start(out=outs[name], in_=ap)

    singles = ctx.enter_context(tc.tile_pool(name="singles", bufs=1))
    big = ctx.enter_context(tc.tile_pool(name="big", bufs=1))
    pipe = ctx.enter_context(tc.tile_pool(name="pipe", bufs=2))
    idxp = ctx.enter_context(tc.tile_pool(name="idxp", bufs=4))
    keep = ctx.enter_context(tc.tile_pool(name="keep", bufs=1))
    gpool = ctx.enter_context(tc.tile_pool(name="gpool", bufs=8))
    wbcp = ctx.enter_context(tc.tile_pool(name="wbcp", bufs=8))
    gmp = ctx.enter_context(tc.tile_pool(name="gmp", bufs=8))
    spool = ctx.enter_context(tc.tile_pool(name="spool", bufs=2))
    psum_big = ctx.enter_context(tc.tile_pool(name="psum_big", bufs=1, space="PSUM"))
    dram = ctx.enter_context(tc.tile_pool(name="dram", bufs=1, space="DRAM"))

    def load(name, shape, dtype):
        t = singles.tile(shape, dtype, tag=f"in_{name}")
        nc.sync.dma_start(out=t[:], in_=ins[name])
        return t

    x_flat = load("x_flat", [128, NPIX_PAD], F32)
    x_bf = load("x_bf", [128, 64 * 66], BF16)  # W-padded [64, 66], center col 1
    hh_dy = load("hh_dy", [128, NT * K], F32)
    ww_dx = load("ww_dx", [128, NT * K], F32)
    w_off_st = load("w_off_st", [128, K * OMCH], BF16)
    w_dc_st = load("w_dc_st", [128, K * 128], BF16)
    b_off = load("b_off_sb", [OMCH, 1], F32)
    b_dc = load("b_dc_sb", [128, 1], F32)
    pos_im = load("pos_im2col", [18, HW], BF16)
    w_pa1 = load("w_pa1_st", [18, 16], BF16)
    w_pa2 = load("w_pa2_st", [16, 128], BF16)
    b_pa1 = load("b_pa1_sb", [16, 1], F32)
    b_pa2 = load("b_pa2_sb", [128, 1], F32)
    ident41 = load("ident41", [OMCH, OMCH], F32)

    # ---- position-attention branch ----
    pa1_ps = psum_big.tile([16, HW], F32, tag="full8")
    for c in range(8):
        nc.tensor.matmul(pa1_ps[:, c * 512:(c + 1) * 512], w_pa1[:],
                         pos_im[:, c * 512:(c + 1) * 512], start=True, stop=True)
    pa1_sb = keep.tile([16, HW], BF16, tag="pa1")
    nc.scalar.activation(out=pa1_sb[:], in_=pa1_ps[:], func=ActFn.Relu, bias=b_pa1[:])

    pa2_ps = psum_big.tile([128, HW], F32, tag="full8")
    for c in range(8):
        nc.tensor.matmul(pa2_ps[:, c * 512:(c + 1) * 512], w_pa2[:],
                         pa1_sb[:, c * 512:(c + 1) * 512], start=True, stop=True)
    pa_sb = keep.tile([128, HW], BF16, tag="pa")
    nc.scalar.activation(out=pa_sb[:], in_=pa2_ps[:], func=ActFn.Sigmoid, bias=b_pa2[:])

    # ---- offset-predictor conv (3x3, pad=1): om[27, 4096] ----
    om_ps = psum_big.tile([OMCH, HW], F32, tag="full8")
    om3 = om_ps[:].rearrange("o (h w) -> o h w", h=H)
    taps = [4] + [k for k in range(9) if k != 4]   # center first: full coverage
    for rg in range(8):
        r0_, r1_ = rg * 8, rg * 8 + 8
        for ki, k in enumerate(taps):
            dy, dx = k // 3 - 1, k % 3 - 1
            h_lo, h_hi = max(r0_, -dy), min(r1_, H - dy)
            if h_lo >= h_hi:
                continue
            nr = h_hi - h_lo
            mov = rsb(x_bf, (h_lo + dy) * 66 + dx + 1, [[66, nr], [1, 64]])
            nc.tensor.matmul(om3[:, h_lo:h_hi, :],
                             w_off_st[:, k * OMCH:(k + 1) * OMCH],
                             mov,
                             start=(ki == 0), stop=(ki == 8),
                             skip_group_check=True)
    om_sb = big.tile([OMCH, HW], F32, tag="big16k")
    nc.scalar.activation(out=om_sb[0:32, :], in_=om_ps[0:32, :],
                         func=ActFn.Relu, bias=b_off[0:32, :])
    nc.scalar.activation(out=om_sb[32:41, :], in_=om_ps[32:41, :],
                         func=ActFn.Relu, bias=b_off[32:41, :])
    nc.scalar.activation(out=om_sb[32:41, :], in_=om_sb[32:41, :],
                         func=ActFn.Sigmoid)

    # ---- transpose om -> pixel-partition omT [128, 32, 27] ----
    omT_ps = psum_big.tile([128, NT, 64], F32, tag="full8")
    for t in range(NT):
        nc.tensor.matmul(omT_ps[:, t, 0:OMCH],
                         om_sb[:, t * 128:(t + 1) * 128], ident41[:],
                         start=True, stop=True)
    omT = big.tile([128, NT, OMCH], F32, tag="omT_sb")
    nc.vector.tensor_copy(out=omT[:], in_=omT_ps[:, :, 0:OMCH])

    dbg("om_sb", om_sb[:])
    dbg("omT", omT[:].rearrange("p t c -> p (t c)"))

    def omt_view(ch_off, ch_step):
        return rsb(omT, ch_off, [[OMCH, NT], [ch_step, K]])

    off_y = omt_view(0, 2)
    off_x = omt_view(1, 2)
    mask_v = omt_view(32, 1)

    P = [128, NT, K]

    def tt(out, a, b, op):
        nc.vector.tensor_tensor(out=out, in0=a, in1=b, op=op)

    def ts(out, a, s1, op0, s2=None, op1=None):
        if op1 is None:
            nc.vector.tensor_scalar(out=out, in0=a, scalar1=s1, scalar2=None,
                                    op0=op0)
        else:
            nc.vector.tensor_scalar(out=out, in0=a, scalar1=s1, scalar2=s2,
                                    op0=op0, op1=op1)

    def axis_pipeline(off_v, base):
        p = pipe.tile(P, F32, tag="coord")
        tt(p[:], off_v, base[:].rearrange("p (t k) -> p t k", t=NT), AluOp.add)
        ci = pipe.tile(P, I16, tag="ci")
        nc.vector.tensor_copy(out=ci[:], in_=p[:])          # cast to int
        cf = pipe.tile(P, F32, tag="cf")
        nc.vector.tensor_copy(out=cf[:], in_=ci[:])         # back to f32
        lt = pipe.tile(P, F32, tag="lt")
        tt(lt[:], p[:], cf[:], AluOp.is_lt)
        fl = pipe.tile(P, F32, tag="fl")
        tt(fl[:], cf[:], lt[:], AluOp.subtract)    # floor = c - (p < c)
        r0 = pipe.tile(P, F32, tag="r0")
        ts(r0[:], fl[:], 0.0, AluOp.max, 63.0, AluOp.min)
        d0 = pipe.tile(P, F32, tag="d0")
        tt(d0[:], p[:], r0[:], AluOp.subtract)
        w0 = pipe.tile(P, F32, tag="w0")
        a0 = pipe.tile(P, F32, tag="a0")
        ts(a0[:], d0[:], -1.0, AluOp.mult, 1.0, AluOp.add)   # 1 - d0
        ts(w0[:], d0[:], 1.0, AluOp.add)                     # 1 + d0
        tt(w0[:], a0[:], w0[:], AluOp.min)
        ts(w0[:], w0[:], 0.0, AluOp.max)           # relu(1-|d0|)
        w1 = pipe.tile(P, F32, tag="w1")
        ts(a0[:], d0[:], -1.0, AluOp.mult, 2.0, AluOp.add)   # 2 - d0
        tt(w1[:], a0[:], d0[:], AluOp.min)
        ts(w1[:], w1[:], 0.0, AluOp.max)           # relu(1-|d0-1|)
        vb = pipe.tile(P, F32, tag="vb")
        ts(vb[:], r0[:], 62.5, AluOp.is_lt)
        tt(w1[:], w1[:], vb[:], AluOp.mult)
        return r0, w0, w1

    r0y, wy0, wy1 = axis_pipeline(off_y, hh_dy)
    r0x, wx0, wx1 = axis_pipeline(off_x, ww_dx)

    mwy0 = pipe.tile(P, F32, tag="mwy0")
    tt(mwy0[:], wy0[:], mask_v, AluOp.mult)
    mwy1 = pipe.tile(P, F32, tag="mwy1")
    tt(mwy1[:], wy1[:], mask_v, AluOp.mult)
    wpl = []
    for ci, (my, wx) in enumerate(((mwy0, wx0), (mwy0, wx1), (mwy1, wx0), (mwy1, wx1))):
        w = keep.tile([128, K, NT], BF16, tag=f"wpl{ci}")
        tt(rsb(w, 0, [[1, NT], [NT, K]]), my[:], wx[:], AluOp.mult)
        wpl.append(w)                               # TL, TR, BL, BR

    e0 = pipe.tile(P, F32, tag="e0")
    ts(e0[:], r0y[:], 64.0, AluOp.mult)
    tt(e0[:], e0[:], r0x[:], AluOp.add)
    idx_pp = []
    for ci, dlt in enumerate((0.0, 1.0, 64.0, 65.0)):
        ii = idxp.tile([128, K, NT], I16, tag="idxpp")
        iiv = rsb(ii, 0, [[1, NT], [NT, K]])
        if dlt == 0.0:
            nc.vector.tensor_copy(out=iiv, in_=e0[:])
        else:
            ts(iiv, e0[:], dlt, AluOp.add)
        idx_pp.append(ii)

    for _c in range(4):
        dbg(f"wpl{_c}", wpl[_c][:].rearrange("p k t -> p (k t)"))
        dbg(f"idxpp{_c}", idx_pp[_c][:].rearrange("p k t -> p (k t)"))

    # ---- bounce idx + weight planes to DRAM in j-order ----
    j2d = [[1, 128], [4096, K], [128, NT]]  # DRAM offset k*4096 + 128t + q
    idx_wr = []
    for c in range(4):
        idram = dram.tile([K * HW], I16, tag=f"idram{c}")
        nc.sync.dma_start(out=rap(idram, 0, j2d), in_=rsb(idx_pp[c], 0, [[1, K * NT]]))
        iw = keep.tile([128, K, HW // 16], I16, tag=f"iw{c}")
        for g in range(8):
            nc.sync.dma_start(
                out=iw[g * 16:(g + 1) * 16, :, :],
                in_=rap(idram, 0, [[1, 16], [HW, K], [16, HW // 16]]))
        idx_wr.append(iw)
    for _c in range(4):
        dbg(f"idxwr{_c}", idx_wr[_c][:].rearrange("p k t -> p (k t)"))
    wdram = []
    for c in range(4):
        wd = dram.tile([K * HW], BF16, tag=f"wdram{c}")
        nc.sync.dma_start(out=rap(wd, 0, j2d), in_=rsb(wpl[c], 0, [[1, K * NT]]))
        wdram.append(wd)

    # ---- main deform loop ----
    out_ps = psum_big.tile([128, HW], F32, tag="full8")
    for k in range(K):
        for ch in range(NCHUNK):
            j0 = ch * CHUNK
            gm = []
            for c in range(4):
                wbc = wbcp.tile([128, CHUNK], BF16, tag="wbc")
                nc.sync.dma_start(
                    out=wbc[:],
                    in_=rap(wdram[c], k * HW + j0, [[0, 128], [1, CHUNK]]))
                g = gpool.tile([128, CHUNK], F32, tag="g")
                if k == 0 and ch == 0:
                    pass
                nc.gpsimd.ap_gather(
                    out_ap=rsb(g, 0, [[1, CHUNK], [1, 1]]),
                    in_ap=rsb(x_flat, 0, [[1, NPIX_PAD], [1, 1]]),
                    idxs_ap=idx_wr[c][:, k, j0 // 16:(j0 + CHUNK) // 16],
                    channels=128, num_elems=NPIX_PAD, d=1, num_idxs=CHUNK)
                if k == 0 and ch == 0:
                    dbg(f"g0_{c}", g[:])
                    dbg(f"wbc0_{c}", wbc[:])
                m = gmp.tile([128, CHUNK], BF16, tag="gm")
                tt(m[:], g[:], wbc[:], AluOp.mult)
                gm.append(m)
            tt(gm[0][:], gm[0][:], gm[1][:], AluOp.add)
            tt(gm[2][:], gm[2][:], gm[3][:], AluOp.add)
            s = spool.tile([128, CHUNK], BF16, tag="s")
            tt(s[:], gm[0][:], gm[2][:], AluOp.add)
            if k == 0 and ch == 0:
                dbg("s00", s[:])
            if "sall" in outs:
                nc.sync.dma_start(
                    out=bass.AP(tensor=outs["sall"].tensor,
                                offset=outs["sall"].offset + k * HW + j0,
                                ap=[[HW * K, 128], [1, CHUNK]]),
                    in_=s[:])
            for q in range(CHUNK // 512):
                nc.tensor.matmul(
                    out_ps[:, j0 + q * 512:j0 + (q + 1) * 512],
                    w_dc_st[:, k * 128:(k + 1) * 128],
                    s[:, q * 512:(q + 1) * 512],
                    start=(k == 0), stop=(k == K - 1), skip_group_check=True)

    dbg("pa_sb", pa_sb[:])
    if "deform" in outs:
        dfm = big.tile([128, HW], F32, tag="big16k")
        nc.vector.tensor_copy(out=dfm[:], in_=out_ps[:])
        nc.sync.dma_start(out=outs["deform"], in_=dfm[:])

    # ---- epilogue: (psum + b_dc) * pa -> out ----
    outf = big.tile([128, HW], F32, tag="big16k")
    nc.vector.scalar_tensor_tensor(
        out=outf[:], in0=out_ps[:], scalar=b_dc[:], in1=pa_sb[:],
        op0=AluOp.add, op1=AluOp.mult)
    nc.sync.dma_start(out=out_dram, in_=outf[:])


# ---------------- host-side preparation ----------------

def _im2col_pos():
    gx = np.linspace(-1.0, 1.0, W, dtype=np.float32)
    gy = np.linspace(-1.0, 1.0, H, dtype=np.float32)
    pos = np.stack([np.broadcast_to(gx[None, :], (H, W)),
                    np.broadcast_to(gy[:, None], (H, W))], 0)  # [2, H, W]
    out = np.zeros((18, HW), np.float32)
    for ci in range(2):
        for k in range(9):
            dy, dx = k // 3 - 1, k % 3 - 1
            sh = np.zeros((H, W), np.float32)
            ys = slice(max(0, -dy), H - max(0, dy))
            xs = slice(max(0, -dx), W - max(0, dx))
            ysrc = slice(max(0, dy), H + min(0, dy))
            xsrc = slice(max(0, dx), W + min(0, dx))
            sh[ys, xs] = pos[ci][ysrc, xsrc]
            out[ci * 9 + k] = sh.reshape(-1)
    return out


def prep_consts(w_off, b_off, w_dc, b_dc, w_pa1, b_pa1, w_pa2, b_pa2):
    q = np.arange(128)
    t = np.arange(NT)
    k = np.arange(K)
    hh = (2 * t[None, :, None] + (q[:, None, None] // 64) + (k[None, None, :] // 3 - 1))
    ww = ((q[:, None, None] % 64) + (k[None, None, :] % 3 - 1))
    hh_dy = np.broadcast_to(hh, (128, NT, K)).astype(np.float32).reshape(128, NT * K)
    ww_dx = np.broadcast_to(ww, (128, NT, K)).astype(np.float32).reshape(128, NT * K)

    OMCH = 41
    w_off_st = np.zeros((128, K * OMCH), BF16_NP)
    for kk in range(K):
        wk = np.zeros((128, OMCH), np.float32)
        wk[:, 0:18] = w_off[0:18, :, kk // 3, kk % 3].T
        wk[:, 32:41] = w_off[18:27, :, kk // 3, kk % 3].T
        w_off_st[:, kk * OMCH:(kk + 1) * OMCH] = wk.astype(BF16_NP)
    w_dc_st = np.zeros((128, K * 128), BF16_NP)
    for kk in range(K):
        blk = np.zeros((128, 128), np.float32)
        for g in range(4):
            # lhsT[c, o] = w_dc[o, c%32, ky, kx] for c,o in group g
            blk[g * 32:(g + 1) * 32, g * 32:(g + 1) * 32] = \
                w_dc[g * 32:(g + 1) * 32, :, kk // 3, kk % 3].T
        w_dc_st[:, kk * 128:(kk + 1) * 128] = blk.astype(BF16_NP)

    w_pa1_st = np.zeros((18, 16), BF16_NP)
    for kk in range(K):
        for ci in range(2):
            w_pa1_st[ci * 9 + kk, :] = w_pa1[:, ci, kk // 3, kk % 3].astype(BF16_NP)
    w_pa2_st = w_pa2[:, :, 0, 0].T.astype(BF16_NP)  # [16, 128]

    return {
        "hh_dy": hh_dy,
        "ww_dx": ww_dx,
        "w_off_st": w_off_st,
        "w_dc_st": w_dc_st,
        "b_off_sb": np.concatenate([b_off[0:18], np.zeros(14, np.float32),
                                    b_off[18:27]]).reshape(41, 1).astype(np.float32),
        "b_dc_sb": b_dc.reshape(128, 1).astype(np.float32),
        "pos_im2col": _im2col_pos().astype(BF16_NP),
        "w_pa1_st": w_pa1_st,
        "w_pa2_st": w_pa2_st,
        "b_pa1_sb": b_pa1.reshape(16, 1).astype(np.float32),
        "b_pa2_sb": b_pa2.reshape(128, 1).astype(np.float32),
        "ident41": np.eye(41, dtype=np.float32),
    }


def prep_sample(x_b):
    """x_b [128, 64, 64] f32 -> per-core input dict."""
    flat = x_b.reshape(128, HW).astype(np.float32)
    x_flat = np.zeros((128, NPIX_PAD), np.float32)
    x_flat[:, :HW] = flat
    xp = np.zeros((128, 64, 66), np.float32)
    xp[:, :, 1:65] = x_b
    return {"x_flat": x_flat, "x_bf": xp.reshape(128, -1).astype(BF16_NP)}


INPUT_SPECS = [
    ("x_flat", [128, NPIX_PAD], F32),
    ("x_bf", [128, 64 * 66], BF16),
    ("hh_dy", [128, NT * K], F32),
    ("ww_dx", [128, NT * K], F32),
    ("w_off_st", [128, K * OMCH], BF16),
    ("w_dc_st", [128, K * 128], BF16),
    ("b_off_sb", [OMCH, 1], F32),
    ("b_dc_sb", [128, 1], F32),
    ("pos_im2col", [18, HW], BF16),
    ("w_pa1_st", [18, 16], BF16),
    ("w_pa2_st", [16, 128], BF16),
    ("b_pa1_sb", [16, 1], F32),
    ("b_pa2_sb", [128, 1], F32),
    ("ident41", [OMCH, OMCH], F32),
]

_CACHE = {}

DBG_SPECS = (
    [("om_sb", [OMCH, HW], F32), ("omT", [128, NT * OMCH], F32),
     ("pa_sb", [128, HW], BF16), ("s00", [128, CHUNK], BF16)]
    + [(f"wpl{c}", [128, K * NT], BF16) for c in range(4)]
    + [(f"idxpp{c}", [128, K * NT], I16) for c in range(4)]
    + [(f"idxwr{c}", [128, K * HW // 16], I16) for c in range(4)]
    + [(f"g0_{c}", [128, CHUNK], F32) for c in range(4)]
    + [(f"wbc0_{c}", [128, CHUNK], BF16) for c in range(4)]
    + [("sall", [128, K * HW], BF16), ("deform", [128, HW], F32)]
)


def build_program(dbg=False):
    key = ("nc", dbg)
    if key in _CACHE:
        return _CACHE[key]
    nc = bacc.Bacc("TRN2", debug=False, num_devices=N_CORES)
    ins = {n: nc.dram_tensor(n, s, d, kind="ExternalInput").ap()
           for n, s, d in INPUT_SPECS}
    outs = {"out": nc.dram_tensor("out", [128, HW], F32, kind="ExternalOutput").ap()}
    if dbg:
        for n, s, d in DBG_SPECS:
            outs[n] = nc.dram_tensor(n, s, d, kind="ExternalOutput").ap()
    with tile.TileContext(nc) as tc:
        dpfa_kernel(tc, outs, ins)
    nc.compile()
    _CACHE[key] = nc
    return nc


def kernel(x, w_off, b_off, w_dc, b_dc, w_pa1, b_pa1, w_pa2, b_pa2, trace=False):
    x = np.asarray(x, dtype=np.float32)
    consts = prep_consts(np.asarray(w_off), np.asarray(b_off), np.asarray(w_dc),
                         np.asarray(b_dc), np.asarray(w_pa1), np.asarray(b_pa1),
                         np.asarray(w_pa2), np.asarray(b_pa2))
    in_maps = []
    for b in range(B):
        m = dict(consts)
        m.update(prep_sample(x[b]))
        in_maps.append(m)
    nc = build_program()
    res = run_bass_kernel_spmd(nc, in_maps, core_ids=list(range(N_CORES)),
                               trace=trace)
    out = np.stack([res.results[b]["out"].reshape(C, H, W) for b in range(B)])
    if trace:
        kernel.last_exec_time_ns = res.exec_time_ns
        kernel.last_results = res
    return out.astype(np.float32)


# revision 23
# speedup vs baseline: 1.6709x; 1.6709x over previous
"""Trainium2 Bass kernel for nn_DPFA_29454885716026 (deformable conv + pos attention).

Strategy: data-parallel over batch B=8 across the 8 NeuronCores (one sample per
core).  Self-contained: builds the Bass/Tile program once (module cache), shards
inputs, runs via bass_utils.run_bass_kernel_spmd, reassembles the full output.

Per-core pipeline (sample x_b [128, 64, 64]):
  PE : offset conv 128->27 (3x3) | om transposes to pixel-partition |
       9-tap block-diag deform matmuls | position-attention matmuls
  ACT: relu/sigmoid evacuations
  DVE: bilinear weight pipeline (pixel-partition) | weight-apply + corner sums
  GPSIMD: ap_gather (4 corners x 9 taps, channel-partition)
  DMA: loads, DRAM bounces for idx/weight layout changes, weight broadcast
"""

import numpy as np
import ml_dtypes
from contextlib import ExitStack

import concourse.bass as bass
import concourse.mybir as mybir
import concourse.tile as tile
from concourse import bacc
from concourse._compat import with_exitstack
from concourse.bass_utils import run_bass_kernel_spmd

F32 = mybir.dt.float32
BF16 = mybir.dt.bfloat16
I16 = mybir.dt.int16
BF16_NP = ml_dtypes.bfloat16

B = 8
C = 128
H = W = 64
HW = H * W
NPIX_PAD = 4224
K = 9
NT = 32
CHUNK = 1024
NCHUNK = HW // CHUNK
AluOp = mybir.AluOpType
ActFn = mybir.ActivationFunctionType

N_CORES = 8
OMCH = 41   # om psum channels: 0-17 offsets, 32-40 masks (18-31 unused)


def rap(t, extra_off, dims):
    """Raw AP (DRAM tiles: all dims are plain strides)."""
    return bass.AP(tensor=t.tensor, offset=t.offset + extra_off,
                   ap=[list(d) for d in dims])


def rsb(t, extra_off, free_dims):
    """SBUF tile view: keep the tile's partition dim, replace free dims."""
    base = t[:]
    return bass.AP(tensor=base.tensor, offset=base.offset + extra_off,
                   ap=[list(base.ap[0])] + [list(d) for d in free_dims])


@with_exitstack
def dpfa_kernel(ctx: ExitStack, tc: tile.TileContext, outs, ins):
    nc = tc.nc
    out_dram = outs["out"]

    def dbg(name, ap):
        if name in outs:
            nc.sync.dma_start(out=outs[name], in_=ap)

    singles = ctx.enter_context(tc.tile_pool(name="singles", bufs=1))
    big = ctx.enter_context(tc.tile_pool(name="big", bufs=1))
    pipe = ctx.enter_context(tc.tile_pool(name="pipe", bufs=2))
    idxp = ctx.enter_context(tc.tile_pool(name="idxp", bufs=4))
    keep = ctx.enter_context(tc.tile_pool(name="keep", bufs=1))
    gpool = ctx.enter_context(tc.tile_pool(name="gpool", bufs=8))
    wbcp = ctx.enter_context(tc.tile_pool(name="wbcp", bufs=2))
    gmp = ctx.enter_context(tc.tile_pool(name="gmp", bufs=8))
    spool = ctx.enter_context(tc.tile_pool(name="spool", bufs=2))
    psum_big = ctx.enter_context(tc.tile_pool(name="psum_big", bufs=1, space="PSUM"))
    dram = ctx.enter_context(tc.tile_pool(name="dram", bufs=1, space="DRAM"))

    def load(name, shape, dtype):
        t = singles.tile(shape, dtype, tag=f"in_{name}")
        nc.sync.dma_start(out=t[:], in_=ins[name])
        return t

    x_flat = load("x_flat", [128, NPIX_PAD], F32)
    x_bf = load("x_bf", [128, 64 * 66], BF16)  # W-padded [64, 66], center col 1
    hh_dy = load("hh_dy", [128, NT * K], F32)
    ww_dx = load("ww_dx", [128, NT * K], F32)
    w_off_st = load("w_off_st", [128, K * OMCH], BF16)
    w_dc_st = load("w_dc_st", [128, K * 128], BF16)
    b_off = load("b_off_sb", [OMCH, 1], F32)
    b_dc = load("b_dc_sb", [128, 1], F32)
    pos_im = load("pos_im2col", [18, HW], BF16)
    w_pa1 = load("w_pa1_st", [18, 16], BF16)
    w_pa2 = load("w_pa2_st", [16, 128], BF16)
    b_pa1 = load("b_pa1_sb", [16, 1], F32)
    b_pa2 = load("b_pa2_sb", [128, 1], F32)
    ident41 = load("ident41", [OMCH, OMCH], F32)

    # ---- position-attention branch ----
    pa1_ps = psum_big.tile([16, HW], F32, tag="full8")
    for c in range(8):
        nc.tensor.matmul(pa1_ps[:, c * 512:(c + 1) * 512], w_pa1[:],
                         pos_im[:, c * 512:(c + 1) * 512], start=True, stop=True)
    pa1_sb = keep.tile([16, HW], BF16, tag="pa1")
    nc.scalar.activation(out=pa1_sb[:], in_=pa1_ps[:], func=ActFn.Relu, bias=b_pa1[:])

    pa2_ps = psum_big.tile([128, HW], F32, tag="full8")
    for c in range(8):
        nc.tensor.matmul(pa2_ps[:, c * 512:(c + 1) * 512], w_pa2[:],
                         pa1_sb[:, c * 512:(c + 1) * 512], start=True, stop=True)
    pa_sb = keep.tile([128, HW], BF16, tag="pa")
    nc.scalar.activation(out=pa_sb[:], in_=pa2_ps[:], func=ActFn.Sigmoid, bias=b_pa2[:])

    # ---- offset-predictor conv (3x3, pad=1): om[27, 4096] ----
    om_ps = psum_big.tile([OMCH, HW], F32, tag="full8")
    om3 = om_ps[:].rearrange("o (h w) -> o h w", h=H)
    taps = [4] + [k for k in range(9) if k != 4]   # center first: full coverage
    for rg in range(8):
        r0_, r1_ = rg * 8, rg * 8 + 8
        for ki, k in enumerate(taps):
            dy, dx = k // 3 - 1, k % 3 - 1
            h_lo, h_hi = max(r0_, -dy), min(r1_, H - dy)
            if h_lo >= h_hi:
                continue
            nr = h_hi - h_lo
            mov = rsb(x_bf, (h_lo + dy) * 66 + dx + 1, [[66, nr], [1, 64]])
            nc.tensor.matmul(om3[:, h_lo:h_hi, :],
                             w_off_st[:, k * OMCH:(k + 1) * OMCH],
                             mov,
                             start=(ki == 0), stop=(ki == 8),
                             skip_group_check=True)
    om_sb = big.tile([OMCH, HW], F32, tag="big16k")
    nc.scalar.activation(out=om_sb[0:32, :], in_=om_ps[0:32, :],
                         func=ActFn.Relu, bias=b_off[0:32, :])
    nc.scalar.activation(out=om_sb[32:41, :], in_=om_ps[32:41, :],
                         func=ActFn.Relu, bias=b_off[32:41, :])
    nc.scalar.activation(out=om_sb[32:41, :], in_=om_sb[32:41, :],
                         func=ActFn.Sigmoid)

    # ---- transpose om -> pixel-partition omT [128, 32, 27] ----
    omT_ps = psum_big.tile([128, NT, 64], F32, tag="full8")
    for t in range(NT):
        nc.tensor.matmul(omT_ps[:, t, 0:OMCH],
                         om_sb[:, t * 128:(t + 1) * 128], ident41[:],
                         start=True, stop=True)
    omT = big.tile([128, NT, OMCH], F32, tag="omT_sb")
    nc.vector.tensor_copy(out=omT[:], in_=omT_ps[:, :, 0:OMCH])

    dbg("om_sb", om_sb[:])
    dbg("omT", omT[:].rearrange("p t c -> p (t c)"))

    def omt_view(ch_off, ch_step):
        return rsb(omT, ch_off, [[OMCH, NT], [ch_step, K]])

    off_y = omt_view(0, 2)
    off_x = omt_view(1, 2)
    mask_v = omt_view(32, 1)

    P = [128, NT, K]

    def tt(out, a, b, op):
        nc.vector.tensor_tensor(out=out, in0=a, in1=b, op=op)

    def ts(out, a, s1, op0, s2=None, op1=None):
        if op1 is None:
            nc.vector.tensor_scalar(out=out, in0=a, scalar1=s1, scalar2=None,
                                    op0=op0)
        else:
            nc.vector.tensor_scalar(out=out, in0=a, scalar1=s1, scalar2=s2,
                                    op0=op0, op1=op1)

    def axis_pipeline(off_v, base):
        p = pipe.tile(P, F32, tag="coord")
        tt(p[:], off_v, base[:].rearrange("p (t k) -> p t k", t=NT), AluOp.add)
        ci = pipe.tile(P, I16, tag="ci")
        nc.vector.tensor_copy(out=ci[:], in_=p[:])          # cast to int
        cf = pipe.tile(P, F32, tag="cf")
        nc.vector.tensor_copy(out=cf[:], in_=ci[:])         # back to f32
        lt = pipe.tile(P, F32, tag="lt")
        tt(lt[:], p[:], cf[:], AluOp.is_lt)
        fl = pipe.tile(P, F32, tag="fl")
        tt(fl[:], cf[:], lt[:], AluOp.subtract)    # floor = c - (p < c)
        r0 = pipe.tile(P, F32, tag="r0")
        ts(r0[:], fl[:], 0.0, AluOp.max, 63.0, AluOp.min)
        d0 = pipe.tile(P, F32, tag="d0")
        tt(d0[:], p[:], r0[:], AluOp.subtract)
        w0 = pipe.tile(P, F32, tag="w0")
        a0 = pipe.tile(P, F32, tag="a0")
        ts(a0[:], d0[:], -1.0, AluOp.mult, 1.0, AluOp.add)   # 1 - d0
        ts(w0[:], d0[:], 1.0, AluOp.add)                     # 1 + d0
        tt(w0[:], a0[:], w0[:], AluOp.min)
        ts(w0[:], w0[:], 0.0, AluOp.max)           # relu(1-|d0|)
        w1 = pipe.tile(P, F32, tag="w1")
        ts(a0[:], d0[:], -1.0, AluOp.mult, 2.0, AluOp.add)   # 2 - d0
        tt(w1[:], a0[:], d0[:], AluOp.min)
        ts(w1[:], w1[:], 0.0, AluOp.max)           # relu(1-|d0-1|)
        vb = pipe.tile(P, F32, tag="vb")
        ts(vb[:], r0[:], 62.5, AluOp.is_lt)
        tt(w1[:], w1[:], vb[:], AluOp.mult)
        return r0, w0, w1

    r0y, wy0, wy1 = axis_pipeline(off_y, hh_dy)
    r0x, wx0, wx1 = axis_pipeline(off_x, ww_dx)

    mwy0 = pipe.tile(P, F32, tag="mwy0")
    tt(mwy0[:], wy0[:], mask_v, AluOp.mult)
    mwy1 = pipe.tile(P, F32, tag="mwy1")
    tt(mwy1[:], wy1[:], mask_v, AluOp.mult)
    # all 4 planes in one tile, free layout (k, pl, t) so per-k cols = (pl, t)
    wpl_all = keep.tile([128, K, 4, NT], BF16, tag="wpl_all")
    for ci, (my, wx) in enumerate(((mwy0, wx0), (mwy0, wx1), (mwy1, wx0), (mwy1, wx1))):
        tt(rsb(wpl_all, ci * NT, [[1, NT], [4 * NT, K]]), my[:], wx[:], AluOp.mult)

    e_pp = []
    for ci, dlt in enumerate((0.0, 1.0, 64.0, 65.0)):
        ee = idxp.tile([128, K * NT], F32, tag="epp")
        ev = rsb(ee, 0, [[1, NT], [NT, K]])     # k-major [k][t] content
        if dlt == 0.0:
            ts(ev, r0y[:], 64.0, AluOp.mult)
            tt(ev, ee[:].rearrange("p (k t) -> p t k", k=K), r0x[:], AluOp.add)
        else:
            ts(ev, e_pp[0][:].rearrange("p (k t) -> p t k", k=K), dlt, AluOp.add)
        e_pp.append(ee)

    # ---- wrapped-16 idx via selection matmuls (iw[p,k,m] = e[16m + p%16]) ----
    sels = [load(f"sel64_{i}", [128, 128], F32) for i in range(4)]
    idx_wr = []
    for c in range(4):
        iw = keep.tile([128, K, HW // 16], I16, tag=f"iw{c}")
        for base in (0, 64):
            for i in range(4):
                q_hi = base // 16 + i
                sps = psum_big.tile([128, K * NT], F32, tag="full8")
                nc.tensor.matmul(sps[:], sels[i][base:base + 64, :],
                                 e_pp[c][base:base + 64, :],
                                 start=True, stop=True)
                nc.vector.tensor_copy(
                    out=rsb(iw, q_hi, [[8 * NT, K], [8, NT]]),
                    in_=sps[:].rearrange("p (k t) -> p k t", k=K))
        idx_wr.append(iw)
    for _c in range(4):
        dbg(f"idxwr{_c}", idx_wr[_c][:].rearrange("p k t -> p (k t)"))

    # ---- weight planes -> DRAM j-order via PE transpose (per k) ----
    ident128 = load("ident128", [128, 128], BF16)
    wt_ps = psum_big.tile([128, K, 128], F32, tag="full8")
    for k in range(K):
        nc.tensor.matmul(wt_ps[:, k, :], wpl_all[:, k, :, :], ident128[:],
                         start=True, stop=True)
    wT_sb = keep.tile([128, K, 128], BF16, tag="wT_sb")
    nc.scalar.activation(out=wT_sb[:], in_=wt_ps[:], func=ActFn.Copy)
    wdram = dram.tile([4, K * HW], BF16, tag="wdram")
    for pl in range(4):
        # src [32 t-part, k, q] -> dram[pl, k*4096 + 128t + q]
        nc.gpsimd.dma_start(
            out=rap(wdram, pl * K * HW, [[128, NT], [4096, K], [1, 128]]),
            in_=wT_sb[pl * 32:(pl + 1) * 32, :, :])

    # ---- main deform loop ----
    out_ps = psum_big.tile([128, HW], F32, tag="full8")
    for k in range(K):
        for ch in range(NCHUNK):
            j0 = ch * CHUNK
            wbc = wbcp.tile([128, 4, CHUNK], BF16, tag="wbc")
            nc.gpsimd.dma_start(
                out=wbc[:],
                in_=rap(wdram, k * HW + j0, [[0, 128], [K * HW, 4], [1, CHUNK]]))
            gm = []
            for c in range(4):
                g = gpool.tile([128, CHUNK], F32, tag="g")
                nc.gpsimd.ap_gather(
                    out_ap=rsb(g, 0, [[1, CHUNK], [1, 1]]),
                    in_ap=rsb(x_flat, 0, [[1, NPIX_PAD], [1, 1]]),
                    idxs_ap=idx_wr[c][:, k, j0 // 16:(j0 + CHUNK) // 16],
                    channels=128, num_elems=NPIX_PAD, d=1, num_idxs=CHUNK)
                if k == 0 and ch == 0:
                    dbg(f"g0_{c}", g[:])
                    dbg(f"wbc0_{c}", wbc[:, c, :])
                m = gmp.tile([128, CHUNK], BF16, tag="gm")
                if c == 3:
                    nc.gpsimd.tensor_tensor(out=m[:], in0=g[:], in1=wbc[:, c, :],
                                            op=AluOp.mult)
                else:
                    tt(m[:], g[:], wbc[:, c, :], AluOp.mult)
                gm.append(m)
            tt(gm[0][:], gm[0][:], gm[1][:], AluOp.add)
            tt(gm[2][:], gm[2][:], gm[3][:], AluOp.add)
            s = spool.tile([128, CHUNK], BF16, tag="s")
            tt(s[:], gm[0][:], gm[2][:], AluOp.add)
            if k == 0 and ch == 0:
                dbg("s00", s[:])
            if "sall" in outs:
                nc.sync.dma_start(
                    out=bass.AP(tensor=outs["sall"].tensor,
                                offset=outs["sall"].offset + k * HW + j0,
                                ap=[[HW * K, 128], [1, CHUNK]]),
                    in_=s[:])
            for q in range(CHUNK // 512):
                nc.tensor.matmul(
                    out_ps[:, j0 + q * 512:j0 + (q + 1) * 512],
                    w_dc_st[:, k * 128:(k + 1) * 128],
                    s[:, q * 512:(q + 1) * 512],
                    start=(k == 0), stop=(k == K - 1), skip_group_check=True)

    dbg("pa_sb", pa_sb[:])
    if "deform" in outs:
        dfm = big.tile([128, HW], F32, tag="big16k")
        nc.vector.tensor_copy(out=dfm[:], in_=out_ps[:])
        nc.sync.dma_start(out=outs["deform"], in_=dfm[:])

    # ---- epilogue: (psum + b_dc) * pa -> out ----
    outf = big.tile([128, HW], F32, tag="big16k")
    nc.vector.scalar_tensor_tensor(
        out=outf[:], in0=out_ps[:], scalar=b_dc[:], in1=pa_sb[:],
        op0=AluOp.add, op1=AluOp.mult)
    nc.sync.dma_start(out=out_dram, in_=outf[:])


# ---------------- host-side preparation ----------------

def _im2col_pos():
    gx = np.linspace(-1.0, 1.0, W, dtype=np.float32)
    gy = np.linspace(-1.0, 1.0, H, dtype=np.float32)
    pos = np.stack([np.broadcast_to(gx[None, :], (H, W)),
                    np.broadcast_to(gy[:, None], (H, W))], 0)  # [2, H, W]
    out = np.zeros((18, HW), np.float32)
    for ci in range(2):
        for k in range(9):
            dy, dx = k // 3 - 1, k % 3 - 1
            sh = np.zeros((H, W), np.float32)
            ys = slice(max(0, -dy), H - max(0, dy))
            xs = slice(max(0, -dx), W - max(0, dx))
            ysrc = slice(max(0, dy), H + min(0, dy))
            xsrc = slice(max(0, dx), W + min(0, dx))
            sh[ys, xs] = pos[ci][ysrc, xsrc]
            out[ci * 9 + k] = sh.reshape(-1)
    return out


def prep_consts(w_off, b_off, w_dc, b_dc, w_pa1, b_pa1, w_pa2, b_pa2):
    q = np.arange(128)
    t = np.arange(NT)
    k = np.arange(K)
    hh = (2 * t[None, :, None] + (q[:, None, None] // 64) + (k[None, None, :] // 3 - 1))
    ww = ((q[:, None, None] % 64) + (k[None, None, :] % 3 - 1))
    hh_dy = np.broadcast_to(hh, (128, NT, K)).astype(np.float32).reshape(128, NT * K)
    ww_dx = np.broadcast_to(ww, (128, NT, K)).astype(np.float32).reshape(128, NT * K)

    OMCH = 41
    w_off_st = np.zeros((128, K * OMCH), BF16_NP)
    for kk in range(K):
        wk = np.zeros((128, OMCH), np.float32)
        wk[:, 0:18] = w_off[0:18, :, kk // 3, kk % 3].T
        wk[:, 32:41] = w_off[18:27, :, kk // 3, kk % 3].T
        w_off_st[:, kk * OMCH:(kk + 1) * OMCH] = wk.astype(BF16_NP)
    w_dc_st = np.zeros((128, K * 128), BF16_NP)
    for kk in range(K):
        blk = np.zeros((128, 128), np.float32)
        for g in range(4):
            # lhsT[c, o] = w_dc[o, c%32, ky, kx] for c,o in group g
            blk[g * 32:(g + 1) * 32, g * 32:(g + 1) * 32] = \
                w_dc[g * 32:(g + 1) * 32, :, kk // 3, kk % 3].T
        w_dc_st[:, kk * 128:(kk + 1) * 128] = blk.astype(BF16_NP)

    w_pa1_st = np.zeros((18, 16), BF16_NP)
    for kk in range(K):
        for ci in range(2):
            w_pa1_st[ci * 9 + kk, :] = w_pa1[:, ci, kk // 3, kk % 3].astype(BF16_NP)
    w_pa2_st = w_pa2[:, :, 0, 0].T.astype(BF16_NP)  # [16, 128]

    return {
        "hh_dy": hh_dy,
        "ww_dx": ww_dx,
        "w_off_st": w_off_st,
        "w_dc_st": w_dc_st,
        "b_off_sb": np.concatenate([b_off[0:18], np.zeros(14, np.float32),
                                    b_off[18:27]]).reshape(41, 1).astype(np.float32),
        "b_dc_sb": b_dc.reshape(128, 1).astype(np.float32),
        "pos_im2col": _im2col_pos().astype(BF16_NP),
        "w_pa1_st": w_pa1_st,
        "w_pa2_st": w_pa2_st,
        "b_pa1_sb": b_pa1.reshape(16, 1).astype(np.float32),
        "b_pa2_sb": b_pa2.reshape(128, 1).astype(np.float32),
        "ident41": np.eye(41, dtype=np.float32),
        "ident128": np.eye(128, dtype=np.float32).astype(BF16_NP),
        **{f"sel64_{i}": np.tile(
            ((np.arange(64)[:, None] % 64) == (i * 16 + np.arange(128) % 16)[None, :]
             ).astype(np.float32), (2, 1)) for i in range(4)},
    }


def prep_sample(x_b):
    """x_b [128, 64, 64] f32 -> per-core input dict."""
    flat = x_b.reshape(128, HW).astype(np.float32)
    x_flat = np.zeros((128, NPIX_PAD), np.float32)
    x_flat[:, :HW] = flat
    xp = np.zeros((128, 64, 66), np.float32)
    xp[:, :, 1:65] = x_b
    return {"x_flat": x_flat, "x_bf": xp.reshape(128, -1).astype(BF16_NP)}


INPUT_SPECS = [
    ("x_flat", [128, NPIX_PAD], F32),
    ("x_bf", [128, 64 * 66], BF16),
    ("hh_dy", [128, NT * K], F32),
    ("ww_dx", [128, NT * K], F32),
    ("w_off_st", [128, K * OMCH], BF16),
    ("w_dc_st", [128, K * 128], BF16),
    ("b_off_sb", [OMCH, 1], F32),
    ("b_dc_sb", [128, 1], F32),
    ("pos_im2col", [18, HW], BF16),
    ("w_pa1_st", [18, 16], BF16),
    ("w_pa2_st", [16, 128], BF16),
    ("b_pa1_sb", [16, 1], F32),
    ("b_pa2_sb", [128, 1], F32),
    ("ident41", [OMCH, OMCH], F32),
    ("ident128", [128, 128], BF16),
    ("sel64_0", [128, 128], F32),
    ("sel64_1", [128, 128], F32),
    ("sel64_2", [128, 128], F32),
    ("sel64_3", [128, 128], F32),
]

_CACHE = {}

DBG_SPECS = (
    [("om_sb", [OMCH, HW], F32), ("omT", [128, NT * OMCH], F32),
     ("pa_sb", [128, HW], BF16), ("s00", [128, CHUNK], BF16)]
    + [(f"idxwr{c}", [128, K * HW // 16], I16) for c in range(4)]
    + [(f"g0_{c}", [128, CHUNK], F32) for c in range(4)]
    + [(f"wbc0_{c}", [128, CHUNK], BF16) for c in range(4)]
    + [("sall", [128, K * HW], BF16), ("deform", [128, HW], F32)]
)


def build_program(dbg=False):
    key = ("nc", dbg)
    if key in _CACHE:
        return _CACHE[key]
    nc = bacc.Bacc("TRN2", debug=False, num_devices=N_CORES)
    ins = {n: nc.dram_tensor(n, s, d, kind="ExternalInput").ap()
           for n, s, d in INPUT_SPECS}
    outs = {"out": nc.dram_tensor("out", [128, HW], F32, kind="ExternalOutput").ap()}
    if dbg:
        for n, s, d in DBG_SPECS:
            outs[n] = nc.dram_tensor(n, s, d, kind="ExternalOutput").ap()
    with tile.TileContext(nc) as tc:
        dpfa_kernel(tc, outs, ins)
    nc.compile()
    _CACHE[key] = nc
    return nc


def kernel(x, w_off, b_off, w_dc, b_dc, w_pa1, b_pa1, w_pa2, b_pa2, trace=False):
    x = np.asarray(x, dtype=np.float32)
    consts = prep_consts(np.asarray(w_off), np.asarray(b_off), np.asarray(w_dc),
                         np.asarray(b_dc), np.asarray(w_pa1), np.asarray(b_pa1),
                         np.asarray(w_pa2), np.asarray(b_pa2))
    in_maps = []
    for b in range(B):
        m = dict(consts)
        m.update(prep_sample(x[b]))
        in_maps.append(m)
    nc = build_program()
    res = run_bass_kernel_spmd(nc, in_maps, core_ids=list(range(N_CORES)),
                               trace=trace)
    out = np.stack([res.results[b]["out"].reshape(C, H, W) for b in range(B)])
    if trace:
        kernel.last_exec_time_ns = res.exec_time_ns
        kernel.last_results = res
    return out.astype(np.float32)


# revision 24
# speedup vs baseline: 1.8478x; 1.1058x over previous
"""Trainium2 Bass kernel for nn_DPFA_29454885716026 (deformable conv + pos attention).

Strategy: data-parallel over batch B=8 across the 8 NeuronCores (one sample per
core).  Self-contained: builds the Bass/Tile program once (module cache), shards
inputs, runs via bass_utils.run_bass_kernel_spmd, reassembles the full output.

Per-core pipeline (sample x_b [128, 64, 64]):
  PE : offset conv 128->27 (3x3) | om transposes to pixel-partition |
       9-tap block-diag deform matmuls | position-attention matmuls
  ACT: relu/sigmoid evacuations
  DVE: bilinear weight pipeline (pixel-partition) | weight-apply + corner sums
  GPSIMD: ap_gather (4 corners x 9 taps, channel-partition)
  DMA: loads, DRAM bounces for idx/weight layout changes, weight broadcast
"""

import numpy as np
import ml_dtypes
from contextlib import ExitStack

import concourse.bass as bass
import concourse.mybir as mybir
import concourse.tile as tile
from concourse import bacc
from concourse._compat import with_exitstack
from concourse.bass_utils import run_bass_kernel_spmd

F32 = mybir.dt.float32
BF16 = mybir.dt.bfloat16
I16 = mybir.dt.int16
BF16_NP = ml_dtypes.bfloat16

B = 8
C = 128
H = W = 64
HW = H * W
NPIX_PAD = 4224
K = 9
NT = 32
CHUNK = 1024
NCHUNK = HW // CHUNK
AluOp = mybir.AluOpType
ActFn = mybir.ActivationFunctionType

N_CORES = 8
OMCH = 41   # om psum channels: 0-17 offsets, 32-40 masks (18-31 unused)


def rap(t, extra_off, dims):
    """Raw AP (DRAM tiles: all dims are plain strides)."""
    return bass.AP(tensor=t.tensor, offset=t.offset + extra_off,
                   ap=[list(d) for d in dims])


def rsb(t, extra_off, free_dims):
    """SBUF tile view: keep the tile's partition dim, replace free dims."""
    base = t[:]
    return bass.AP(tensor=base.tensor, offset=base.offset + extra_off,
                   ap=[list(base.ap[0])] + [list(d) for d in free_dims])


@with_exitstack
def dpfa_kernel(ctx: ExitStack, tc: tile.TileContext, outs, ins):
    nc = tc.nc
    out_dram = outs["out"]

    def dbg(name, ap):
        if name in outs:
            nc.sync.dma_start(out=outs[name], in_=ap)

    singles = ctx.enter_context(tc.tile_pool(name="singles", bufs=1))
    big = ctx.enter_context(tc.tile_pool(name="big", bufs=1))
    pipe = ctx.enter_context(tc.tile_pool(name="pipe", bufs=1))
    pipe2 = ctx.enter_context(tc.tile_pool(name="pipe2", bufs=2))
    idxp = ctx.enter_context(tc.tile_pool(name="idxp", bufs=4))
    keep = ctx.enter_context(tc.tile_pool(name="keep", bufs=1))
    gpool = ctx.enter_context(tc.tile_pool(name="gpool", bufs=4))
    wbcp = ctx.enter_context(tc.tile_pool(name="wbcp", bufs=2))
    psum_big = ctx.enter_context(tc.tile_pool(name="psum_big", bufs=1, space="PSUM"))
    dram = ctx.enter_context(tc.tile_pool(name="dram", bufs=1, space="DRAM"))

    def load(name, shape, dtype):
        t = singles.tile(shape, dtype, tag=f"in_{name}")
        nc.sync.dma_start(out=t[:], in_=ins[name])
        return t

    x_flat = load("x_flat", [128, NPIX_PAD], F32)
    x_bf = load("x_bf", [128, 64 * 66], BF16)  # W-padded [64, 66], center col 1
    hh_dy = load("hh_dy", [128, NT * K], F32)
    ww_dx = load("ww_dx", [128, NT * K], F32)
    w_off_st = load("w_off_st", [128, K * OMCH], BF16)
    w_dc_st = load("w_dc_st", [128, K * 128], BF16)
    b_off = load("b_off_sb", [OMCH, 1], F32)
    b_dc = load("b_dc_sb", [128, 1], F32)
    pos_im = load("pos_im2col", [18, HW], BF16)
    w_pa1 = load("w_pa1_st", [18, 16], BF16)
    w_pa2 = load("w_pa2_st", [16, 128], BF16)
    b_pa1 = load("b_pa1_sb", [16, 1], F32)
    b_pa2 = load("b_pa2_sb", [128, 1], F32)
    ident41 = load("ident41", [OMCH, OMCH], F32)

    # ---- position-attention branch ----
    pa1_ps = psum_big.tile([16, HW], F32, tag="full8")
    for c in range(8):
        nc.tensor.matmul(pa1_ps[:, c * 512:(c + 1) * 512], w_pa1[:],
                         pos_im[:, c * 512:(c + 1) * 512], start=True, stop=True)
    pa1_sb = keep.tile([16, HW], BF16, tag="pa1")
    nc.scalar.activation(out=pa1_sb[:], in_=pa1_ps[:], func=ActFn.Relu, bias=b_pa1[:])

    pa2_ps = psum_big.tile([128, HW], F32, tag="full8")
    for c in range(8):
        nc.tensor.matmul(pa2_ps[:, c * 512:(c + 1) * 512], w_pa2[:],
                         pa1_sb[:, c * 512:(c + 1) * 512], start=True, stop=True)
    pa_sb = keep.tile([128, HW], BF16, tag="pa")
    nc.scalar.activation(out=pa_sb[:], in_=pa2_ps[:], func=ActFn.Sigmoid, bias=b_pa2[:])

    # ---- offset-predictor conv (3x3, pad=1): om[27, 4096] ----
    om_ps = psum_big.tile([OMCH, HW], F32, tag="full8")
    om3 = om_ps[:].rearrange("o (h w) -> o h w", h=H)
    taps = [4] + [k for k in range(9) if k != 4]   # center first: full coverage
    for rg in range(8):
        r0_, r1_ = rg * 8, rg * 8 + 8
        for ki, k in enumerate(taps):
            dy, dx = k // 3 - 1, k % 3 - 1
            h_lo, h_hi = max(r0_, -dy), min(r1_, H - dy)
            if h_lo >= h_hi:
                continue
            nr = h_hi - h_lo
            mov = rsb(x_bf, (h_lo + dy) * 66 + dx + 1, [[66, nr], [1, 64]])
            nc.tensor.matmul(om3[:, h_lo:h_hi, :],
                             w_off_st[:, k * OMCH:(k + 1) * OMCH],
                             mov,
                             start=(ki == 0), stop=(ki == 8),
                             skip_group_check=True)
    om_sb = big.tile([OMCH, HW], F32, tag="big16k")
    nc.scalar.activation(out=om_sb[0:32, :], in_=om_ps[0:32, :],
                         func=ActFn.Relu, bias=b_off[0:32, :])
    nc.scalar.activation(out=om_sb[32:41, :], in_=om_ps[32:41, :],
                         func=ActFn.Relu, bias=b_off[32:41, :])
    nc.scalar.activation(out=om_sb[32:41, :], in_=om_sb[32:41, :],
                         func=ActFn.Sigmoid)

    # ---- transpose om -> pixel-partition omT [128, 32, 27] ----
    omT_ps = psum_big.tile([128, NT, 64], F32, tag="full8")
    for t in range(NT):
        nc.tensor.matmul(omT_ps[:, t, 0:OMCH],
                         om_sb[:, t * 128:(t + 1) * 128], ident41[:],
                         start=True, stop=True)
    omT = big.tile([128, NT, OMCH], F32, tag="omT_sb")
    nc.vector.tensor_copy(out=omT[:], in_=omT_ps[:, :, 0:OMCH])

    dbg("om_sb", om_sb[:])
    dbg("omT", omT[:].rearrange("p t c -> p (t c)"))

    def omt_view(ch_off, ch_step):
        return rsb(omT, ch_off, [[OMCH, NT], [ch_step, K]])

    off_y = omt_view(0, 2)
    off_x = omt_view(1, 2)
    mask_v = omt_view(32, 1)

    P = [128, NT, K]

    def tt(out, a, b, op):
        nc.vector.tensor_tensor(out=out, in0=a, in1=b, op=op)

    def ts(out, a, s1, op0, s2=None, op1=None):
        if op1 is None:
            nc.vector.tensor_scalar(out=out, in0=a, scalar1=s1, scalar2=None,
                                    op0=op0)
        else:
            nc.vector.tensor_scalar(out=out, in0=a, scalar1=s1, scalar2=s2,
                                    op0=op0, op1=op1)

    def axis_pipeline(off_v, base):
        p = pipe.tile(P, F32, tag="coord")
        tt(p[:], off_v, base[:].rearrange("p (t k) -> p t k", t=NT), AluOp.add)
        ci = pipe.tile(P, I16, tag="ci")
        nc.vector.tensor_copy(out=ci[:], in_=p[:])          # cast to int
        cf = pipe.tile(P, F32, tag="cf")
        nc.vector.tensor_copy(out=cf[:], in_=ci[:])         # back to f32
        lt = pipe.tile(P, F32, tag="lt")
        tt(lt[:], p[:], cf[:], AluOp.is_lt)
        fl = pipe.tile(P, F32, tag="fl")
        tt(fl[:], cf[:], lt[:], AluOp.subtract)    # floor = c - (p < c)
        r0 = pipe2.tile(P, F32, tag="r0")
        ts(r0[:], fl[:], 0.0, AluOp.max, 63.0, AluOp.min)
        d0 = pipe.tile(P, F32, tag="d0")
        tt(d0[:], p[:], r0[:], AluOp.subtract)
        w0 = pipe2.tile(P, F32, tag="w0")
        a0 = pipe.tile(P, F32, tag="a0")
        ts(a0[:], d0[:], -1.0, AluOp.mult, 1.0, AluOp.add)   # 1 - d0
        ts(w0[:], d0[:], 1.0, AluOp.add)                     # 1 + d0
        tt(w0[:], a0[:], w0[:], AluOp.min)
        ts(w0[:], w0[:], 0.0, AluOp.max)           # relu(1-|d0|)
        w1 = pipe2.tile(P, F32, tag="w1")
        ts(a0[:], d0[:], -1.0, AluOp.mult, 2.0, AluOp.add)   # 2 - d0
        tt(w1[:], a0[:], d0[:], AluOp.min)
        ts(w1[:], w1[:], 0.0, AluOp.max)           # relu(1-|d0-1|)
        vb = pipe.tile(P, F32, tag="vb")
        ts(vb[:], r0[:], 62.5, AluOp.is_lt)
        tt(w1[:], w1[:], vb[:], AluOp.mult)
        return r0, w0, w1

    r0y, wy0, wy1 = axis_pipeline(off_y, hh_dy)
    r0x, wx0, wx1 = axis_pipeline(off_x, ww_dx)

    mwy0 = pipe.tile(P, F32, tag="mwy0")
    tt(mwy0[:], wy0[:], mask_v, AluOp.mult)
    mwy1 = pipe.tile(P, F32, tag="mwy1")
    tt(mwy1[:], wy1[:], mask_v, AluOp.mult)
    # all 4 planes in one tile, free layout (k, pl, t) so per-k cols = (pl, t)
    wpl_all = keep.tile([128, K, 4, NT], BF16, tag="wpl_all")
    for ci, (my, wx) in enumerate(((mwy0, wx0), (mwy0, wx1), (mwy1, wx0), (mwy1, wx1))):
        tt(rsb(wpl_all, ci * NT, [[1, NT], [4 * NT, K]]), my[:], wx[:], AluOp.mult)

    e_pp = []
    for ci, dlt in enumerate((0.0, 1.0, 64.0, 65.0)):
        ee = idxp.tile([128, K * NT], F32, tag="epp")
        ev = rsb(ee, 0, [[1, NT], [NT, K]])     # k-major [k][t] content
        if dlt == 0.0:
            ts(ev, r0y[:], 64.0, AluOp.mult)
            tt(ev, ee[:].rearrange("p (k t) -> p t k", k=K), r0x[:], AluOp.add)
        else:
            ts(ev, e_pp[0][:].rearrange("p (k t) -> p t k", k=K), dlt, AluOp.add)
        e_pp.append(ee)

    # ---- wrapped-16 idx via selection matmuls (iw[p,k,m] = e[16m + p%16]) ----
    sels = [load(f"sel64_{i}", [128, 128], F32) for i in range(4)]
    idx_wr = []
    for c in range(4):
        iw = keep.tile([128, K, HW // 16], I16, tag=f"iw{c}")
        for base in (0, 64):
            for i in range(4):
                q_hi = base // 16 + i
                sps = psum_big.tile([128, K * NT], F32, tag="full8")
                nc.tensor.matmul(sps[:], sels[i][base:base + 64, :],
                                 e_pp[c][base:base + 64, :],
                                 start=True, stop=True)
                nc.vector.tensor_copy(
                    out=rsb(iw, q_hi, [[8 * NT, K], [8, NT]]),
                    in_=sps[:].rearrange("p (k t) -> p k t", k=K))
        idx_wr.append(iw)
    for _c in range(4):
        dbg(f"idxwr{_c}", idx_wr[_c][:].rearrange("p k t -> p (k t)"))

    # ---- weight planes -> DRAM j-order via PE transpose (per k) ----
    ident128 = load("ident128", [128, 128], BF16)
    wt_ps = psum_big.tile([128, K, 128], F32, tag="full8")
    for k in range(K):
        nc.tensor.matmul(wt_ps[:, k, :], wpl_all[:, k, :, :], ident128[:],
                         start=True, stop=True)
    wT_sb = keep.tile([128, K, 128], BF16, tag="wT_sb")
    nc.scalar.activation(out=wT_sb[:], in_=wt_ps[:], func=ActFn.Copy)
    wdram = dram.tile([4, K * HW], BF16, tag="wdram")
    for pl in range(4):
        # src [32 t-part, k, q] -> dram[pl, k*4096 + 128t + q]
        nc.gpsimd.dma_start(
            out=rap(wdram, pl * K * HW, [[128, NT], [4096, K], [1, 128]]),
            in_=wT_sb[pl * 32:(pl + 1) * 32, :, :])

    # ---- main deform loop ----
    out_ps = psum_big.tile([128, HW], F32, tag="full8")
    for k in range(K):
        for ch in range(NCHUNK):
            j0 = ch * CHUNK
            wbc = wbcp.tile([128, 4, CHUNK], BF16, tag="wbc")
            nc.gpsimd.dma_start(
                out=wbc[:],
                in_=rap(wdram, k * HW + j0, [[0, 128], [K * HW, 4], [1, CHUNK]]))
            gm = []
            for c in range(4):
                g = gpool.tile([128, CHUNK], F32, tag="g")
                nc.gpsimd.ap_gather(
                    out_ap=rsb(g, 0, [[1, CHUNK], [1, 1]]),
                    in_ap=rsb(x_flat, 0, [[1, NPIX_PAD], [1, 1]]),
                    idxs_ap=idx_wr[c][:, k, j0 // 16:(j0 + CHUNK) // 16],
                    channels=128, num_elems=NPIX_PAD, d=1, num_idxs=CHUNK)
                if k == 0 and ch == 0:
                    dbg(f"g0_{c}", g[:])
                    dbg(f"wbc0_{c}", wbc[:, c, :])
                # in-place: write bf16 product into the low half of g's f32 slot
                m = g[:].bitcast(BF16)[:, 0:CHUNK]
                tt(m, g[:], wbc[:, c, :], AluOp.mult)
                gm.append(m)
            tt(gm[0], gm[0], gm[1], AluOp.add)
            tt(gm[2], gm[2], gm[3], AluOp.add)
            s = gm[0]
            tt(s, gm[0], gm[2], AluOp.add)
            if k == 0 and ch == 0:
                dbg("s00", s)
            if "sall" in outs:
                nc.sync.dma_start(
                    out=bass.AP(tensor=outs["sall"].tensor,
                                offset=outs["sall"].offset + k * HW + j0,
                                ap=[[HW * K, 128], [1, CHUNK]]),
                    in_=s)
            for q in range(CHUNK // 512):
                nc.tensor.matmul(
                    out_ps[:, j0 + q * 512:j0 + (q + 1) * 512],
                    w_dc_st[:, k * 128:(k + 1) * 128],
                    s[:, q * 512:(q + 1) * 512],
                    start=(k == 0), stop=(k == K - 1), skip_group_check=True)

    dbg("pa_sb", pa_sb[:])
    if "deform" in outs:
        dfm = big.tile([128, HW], F32, tag="big16k")
        nc.vector.tensor_copy(out=dfm[:], in_=out_ps[:])
        nc.sync.dma_start(out=outs["deform"], in_=dfm[:])

    # ---- epilogue: (psum + b_dc) * pa -> out ----
    outf = big.tile([128, HW], F32, tag="big16k")
    nc.vector.scalar_tensor_tensor(
        out=outf[:], in0=out_ps[:], scalar=b_dc[:], in1=pa_sb[:],
        op0=AluOp.add, op1=AluOp.mult)
    nc.sync.dma_start(out=out_dram, in_=outf[:])


# ---------------- host-side preparation ----------------

def _im2col_pos():
    gx = np.linspace(-1.0, 1.0, W, dtype=np.float32)
    gy = np.linspace(-1.0, 1.0, H, dtype=np.float32)
    pos = np.stack([np.broadcast_to(gx[None, :], (H, W)),
                    np.broadcast_to(gy[:, None], (H, W))], 0)  # [2, H, W]
    out = np.zeros((18, HW), np.float32)
    for ci in range(2):
        for k in range(9):
            dy, dx = k // 3 - 1, k % 3 - 1
            sh = np.zeros((H, W), np.float32)
            ys = slice(max(0, -dy), H - max(0, dy))
            xs = slice(max(0, -dx), W - max(0, dx))
            ysrc = slice(max(0, dy), H + min(0, dy))
            xsrc = slice(max(0, dx), W + min(0, dx))
            sh[ys, xs] = pos[ci][ysrc, xsrc]
            out[ci * 9 + k] = sh.reshape(-1)
    return out


def prep_consts(w_off, b_off, w_dc, b_dc, w_pa1, b_pa1, w_pa2, b_pa2):
    q = np.arange(128)
    t = np.arange(NT)
    k = np.arange(K)
    hh = (2 * t[None, :, None] + (q[:, None, None] // 64) + (k[None, None, :] // 3 - 1))
    ww = ((q[:, None, None] % 64) + (k[None, None, :] % 3 - 1))
    hh_dy = np.broadcast_to(hh, (128, NT, K)).astype(np.float32).reshape(128, NT * K)
    ww_dx = np.broadcast_to(ww, (128, NT, K)).astype(np.float32).reshape(128, NT * K)

    OMCH = 41
    w_off_st = np.zeros((128, K * OMCH), BF16_NP)
    for kk in range(K):
        wk = np.zeros((128, OMCH), np.float32)
        wk[:, 0:18] = w_off[0:18, :, kk // 3, kk % 3].T
        wk[:, 32:41] = w_off[18:27, :, kk // 3, kk % 3].T
        w_off_st[:, kk * OMCH:(kk + 1) * OMCH] = wk.astype(BF16_NP)
    w_dc_st = np.zeros((128, K * 128), BF16_NP)
    for kk in range(K):
        blk = np.zeros((128, 128), np.float32)
        for g in range(4):
            # lhsT[c, o] = w_dc[o, c%32, ky, kx] for c,o in group g
            blk[g * 32:(g + 1) * 32, g * 32:(g + 1) * 32] = \
                w_dc[g * 32:(g + 1) * 32, :, kk // 3, kk % 3].T
        w_dc_st[:, kk * 128:(kk + 1) * 128] = blk.astype(BF16_NP)

    w_pa1_st = np.zeros((18, 16), BF16_NP)
    for kk in range(K):
        for ci in range(2):
            w_pa1_st[ci * 9 + kk, :] = w_pa1[:, ci, kk // 3, kk % 3].astype(BF16_NP)
    w_pa2_st = w_pa2[:, :, 0, 0].T.astype(BF16_NP)  # [16, 128]

    return {
        "hh_dy": hh_dy,
        "ww_dx": ww_dx,
        "w_off_st": w_off_st,
        "w_dc_st": w_dc_st,
        "b_off_sb": np.concatenate([b_off[0:18], np.zeros(14, np.float32),
                                    b_off[18:27]]).reshape(41, 1).astype(np.float32),
        "b_dc_sb": b_dc.reshape(128, 1).astype(np.float32),
        "pos_im2col": _im2col_pos().astype(BF16_NP),
        "w_pa1_st": w_pa1_st,
        "w_pa2_st": w_pa2_st,
        "b_pa1_sb": b_pa1.reshape(16, 1).astype(np.float32),
        "b_pa2_sb": b_pa2.reshape(128, 1).astype(np.float32),
        "ident41": np.eye(41, dtype=np.float32),
        "ident128": np.eye(128, dtype=np.float32).astype(BF16_NP),
        **{f"sel64_{i}": np.tile(
            ((np.arange(64)[:, None] % 64) == (i * 16 + np.arange(128) % 16)[None, :]
             ).astype(np.float32), (2, 1)) for i in range(4)},
    }


def prep_sample(x_b):
    """x_b [128, 64, 64] f32 -> per-core input dict."""
    flat = x_b.reshape(128, HW).astype(np.float32)
    x_flat = np.zeros((128, NPIX_PAD), np.float32)
    x_flat[:, :HW] = flat
    xp = np.zeros((128, 64, 66), np.float32)
    xp[:, :, 1:65] = x_b
    return {"x_flat": x_flat, "x_bf": xp.reshape(128, -1).astype(BF16_NP)}


INPUT_SPECS = [
    ("x_flat", [128, NPIX_PAD], F32),
    ("x_bf", [128, 64 * 66], BF16),
    ("hh_dy", [128, NT * K], F32),
    ("ww_dx", [128, NT * K], F32),
    ("w_off_st", [128, K * OMCH], BF16),
    ("w_dc_st", [128, K * 128], BF16),
    ("b_off_sb", [OMCH, 1], F32),
    ("b_dc_sb", [128, 1], F32),
    ("pos_im2col", [18, HW], BF16),
    ("w_pa1_st", [18, 16], BF16),
    ("w_pa2_st", [16, 128], BF16),
    ("b_pa1_sb", [16, 1], F32),
    ("b_pa2_sb", [128, 1], F32),
    ("ident41", [OMCH, OMCH], F32),
    ("ident128", [128, 128], BF16),
    ("sel64_0", [128, 128], F32),
    ("sel64_1", [128, 128], F32),
    ("sel64_2", [128, 128], F32),
    ("sel64_3", [128, 128], F32),
]

_CACHE = {}

DBG_SPECS = (
    [("om_sb", [OMCH, HW], F32), ("omT", [128, NT * OMCH], F32),
     ("pa_sb", [128, HW], BF16), ("s00", [128, CHUNK], BF16)]
    + [(f"idxwr{c}", [128, K * HW // 16], I16) for c in range(4)]
    + [(f"g0_{c}", [128, CHUNK], F32) for c in range(4)]
    + [(f"wbc0_{c}", [128, CHUNK], BF16) for c in range(4)]
    + [("sall", [128, K * HW], BF16), ("deform", [128, HW], F32)]
)


def build_program(dbg=False):
    key = ("nc", dbg)
    if key in _CACHE:
        return _CACHE[key]
    nc = bacc.Bacc("TRN2", debug=False, num_devices=N_CORES)
    ins = {n: nc.dram_tensor(n, s, d, kind="ExternalInput").ap()
           for n, s, d in INPUT_SPECS}
    outs = {"out": nc.dram_tensor("out", [128, HW], F32, kind="ExternalOutput").ap()}
    if dbg:
        for n, s, d in DBG_SPECS:
            outs[n] = nc.dram_tensor(n, s, d, kind="ExternalOutput").ap()
    with tile.TileContext(nc) as tc:
        dpfa_kernel(tc, outs, ins)
    nc.compile()
    _CACHE[key] = nc
    return nc


def kernel(x, w_off, b_off, w_dc, b_dc, w_pa1, b_pa1, w_pa2, b_pa2, trace=False):
    x = np.asarray(x, dtype=np.float32)
    consts = prep_consts(np.asarray(w_off), np.asarray(b_off), np.asarray(w_dc),
                         np.asarray(b_dc), np.asarray(w_pa1), np.asarray(b_pa1),
                         np.asarray(w_pa2), np.asarray(b_pa2))
    in_maps = []
    for b in range(B):
        m = dict(consts)
        m.update(prep_sample(x[b]))
        in_maps.append(m)
    nc = build_program()
    res = run_bass_kernel_spmd(nc, in_maps, core_ids=list(range(N_CORES)),
                               trace=trace)
    out = np.stack([res.results[b]["out"].reshape(C, H, W) for b in range(B)])
    if trace:
        kernel.last_exec_time_ns = res.exec_time_ns
        kernel.last_results = res
    return out.astype(np.float32)
